# revision 13
# baseline (speedup 1.0000x reference)
"""GNN message passing on 8 Trainium2 NeuronCores.

Reference computation:
    h = x @ W                       # [N, D]
    msg = h[src]                    # [E, D]
    out = relu(segment_sum(msg, dst, N))

Key identity: segment_sum(x[src]) @ W == segment_sum(x[src] @ W), so we
aggregate raw x rows and apply the small 128x128 matmul once per output
window at the end.

Distribution: shard edges by DESTINATION block (6250 dsts per core), so
outputs are disjoint and no all-reduce is needed. Each core:
  1. dma_gather's bf16 x rows for its edges from a replicated node table
     in HBM (split lo/hi so gather indices fit in int16),
  2. scatters each 128-slot tile into per-window PSUM accumulators via
     TensorE matmuls against one-hot selection matrices built on-device
     with one DVE compare (iota == dloc) per window,
  3. applies the final @W matmul + relu per window and DMAs out fp32.

The dma_gather descriptor pipeline costs ~5-6ns/index regardless of
source or payload size (gpsimd ucode serializes index unwrap +
descriptor generation), so gather cost scales with slot count.  To
minimize slots the host BALANCES the dst->window assignment per core
(greedy bin packing on per-dst lo/hi edge counts): every window fits
t_lo=8 lo tiles, and hi capacity alternates by chunk (chunks 0-3 get 5
hi tiles per window, chunks 4-6 get 4) with hi-heavy dsts packed into
the roomier windows.  The host unpermutes the output rows at the end.
"""

import numpy as np

P = 128
D = 128
N_NODES = 50000
N_CORES = 8
NW = 49                 # 128-dst windows per core
DPC = N_NODES // N_CORES        # real dsts per core = 6250
CG = 7                  # windows per gather chunk (NW % CG == 0)
NCH = NW // CG
LO_CUT = 32767          # src < LO_CUT -> lo table (idx = src+1 <= 32767)
LO_ROWS = LO_CUT + 1
HI_ROWS = N_NODES - LO_CUT + 1
CAP_LO = 8              # lo tiles per window
TH_DEFAULT = (5, 5, 5, 5, 4, 4, 4)   # hi tiles per window, by chunk

_NC_CACHE = {}


def _hi_tiles(th):
    """Per-window hi tile counts and tile-offset prefix for pattern th."""
    t_w = np.repeat(np.asarray(th, np.int64), CG)          # [NW]
    off = np.concatenate([[0], np.cumsum(t_w)[:-1]])       # tile offset per window
    return t_w, off


def _build_nc(t_lo, th, nw=NW, cg=CG, lo_rows=LO_ROWS, hi_rows=HI_ROWS,
              bench_reps=1, parts="all", dma_scratch=16384, n_queues=2):
    key = (t_lo, tuple(th), nw, cg, lo_rows, hi_rows, bench_reps, parts,
           dma_scratch, n_queues, "v3")
    if key in _NC_CACHE:
        return _NC_CACHE[key]

    import concourse.bacc as bacc
    import concourse.mybir as mybir
    import concourse.tile as tile
    from concourse import library_config

    assert nw % cg == 0
    nch = nw // cg
    t_hi_w, ho_tiles = _hi_tiles(th)
    n_lo = nw * t_lo * P
    hi_tiles_total = int(t_hi_w.sum())
    n_hi = hi_tiles_total * P
    th_max = max(th)
    tmax = max(t_lo, th_max)

    nc = bacc.Bacc(
        "TRN2", target_bir_lowering=False, debug=False, num_swdge_queues=n_queues,
        dynamic_dma_scratch_size=dma_scratch,
    )
    f32 = mybir.dt.float32
    bf16 = mybir.dt.bfloat16
    tbl_lo = nc.dram_tensor("tbl_lo", [lo_rows, D], bf16, kind="ExternalInput")
    tbl_hi = nc.dram_tensor("tbl_hi", [hi_rows, D], bf16, kind="ExternalInput")
    idx_lo = nc.dram_tensor(
        "idx_lo", [P, n_lo // 16], mybir.dt.int16, kind="ExternalInput"
    )
    idx_hi = nc.dram_tensor(
        "idx_hi", [P, n_hi // 16], mybir.dt.int16, kind="ExternalInput"
    )
    dloc_lo = nc.dram_tensor("dloc_lo", [P, nw * t_lo], bf16, kind="ExternalInput")
    dloc_hi = nc.dram_tensor("dloc_hi", [P, hi_tiles_total], bf16,
                             kind="ExternalInput")
    wmat = nc.dram_tensor("wmat", [D, D], bf16, kind="ExternalInput")
    iota = nc.dram_tensor("iota", [P, tmax * P], bf16, kind="ExternalInput")
    out = nc.dram_tensor("out", [nw * P, D], f32, kind="ExternalOutput")
    if parts == "compute":
        dummy_a = nc.dram_tensor(
            "dummy_a", [P, cg * t_lo, P], bf16, kind="ExternalInput"
        )
        dummy_b = nc.dram_tensor(
            "dummy_b", [P, cg * th_max, P], bf16, kind="ExternalInput"
        )

    with tile.TileContext(nc) as tc:
        nc.gpsimd.load_library(library_config.mlp)
        with (
            tc.tile_pool(name="const", bufs=1) as cpool,
            tc.tile_pool(name="msga", bufs=3) as apool,
            tc.tile_pool(name="msgb", bufs=3) as bpool,
            tc.tile_pool(name="sel", bufs=6) as spool,
            tc.tile_pool(name="agg", bufs=4) as gpool,
            tc.tile_pool(name="outp", bufs=4) as opool,
            tc.tile_pool(name="psw", bufs=4, space="PSUM") as pwpool,
            tc.tile_pool(name="pso", bufs=2, space="PSUM") as popool,
        ):
            w_sb = cpool.tile([D, D], bf16, tag="w")
            nc.sync.dma_start(out=w_sb[:], in_=wmat.ap())
            iota_sb = cpool.tile([P, tmax, P], bf16, tag="iota")
            nc.sync.dma_start(
                out=iota_sb[:], in_=iota.ap().rearrange("p (t j) -> p t j", t=tmax)
            )
            il_sb = cpool.tile([P, n_lo // 16], mybir.dt.int16, tag="il")
            nc.sync.dma_start(out=il_sb[:], in_=idx_lo.ap())
            ih_sb = cpool.tile([P, n_hi // 16], mybir.dt.int16, tag="ih")
            nc.sync.dma_start(out=ih_sb[:], in_=idx_hi.ap())
            dl_sb = cpool.tile([P, nw * t_lo], bf16, tag="dl")
            nc.sync.dma_start(out=dl_sb[:], in_=dloc_lo.ap())
            dh_sb = cpool.tile([P, hi_tiles_total], bf16, tag="dh")
            nc.sync.dma_start(out=dh_sb[:], in_=dloc_hi.ap())

            def body():
              for ch in range(nch):
                t_hi = th[ch]
                a_tile = apool.tile([P, cg * t_lo, D], bf16, tag="msga")
                b_tile = bpool.tile([P, cg * t_hi, D], bf16, tag="msgb")
                if parts in ("all", "gather"):
                    _emit_gathers(ch, t_hi, a_tile, b_tile)
                if parts == "compute":
                    nc.sync.dma_start(out=a_tile[:], in_=dummy_a.ap())
                    nc.sync.dma_start(
                        out=b_tile[:], in_=dummy_b.ap()[:, 0 : cg * t_hi, :]
                    )
                if parts in ("all", "compute"):
                    _emit_compute(ch, t_hi, a_tile, b_tile)
                if parts == "gather":
                    psw = pwpool.tile([P, P], f32, tag="psw")
                    nc.tensor.matmul(
                        psw[:], a_tile[:, 0, :], b_tile[:, 0, :],
                        start=True, stop=True,
                    )
                    o_sb = opool.tile([P, D], f32, tag="out")
                    nc.scalar.copy(o_sb[:], psw[:])
                    nc.sync.dma_start(out=out.ap()[0:P, :], in_=o_sb[:])

            def _emit_gathers(ch, t_hi, a_tile, b_tile):
                nc.gpsimd.dma_gather(
                    a_tile[:],
                    tbl_lo.ap(),
                    il_sb[:, ch * cg * t_lo * 8 : (ch + 1) * cg * t_lo * 8],
                    cg * t_lo * P,
                    cg * t_lo * P,
                    D,
                    queue_num=0,
                    single_packet=False,
                )
                hi_tile_base = int(ho_tiles[ch * cg])
                nc.gpsimd.dma_gather(
                    b_tile[:],
                    tbl_hi.ap(),
                    ih_sb[:, hi_tile_base * 8 : (hi_tile_base + cg * t_hi) * 8],
                    cg * t_hi * P,
                    cg * t_hi * P,
                    D,
                    queue_num=1 % n_queues,
                    single_packet=False,
                )

            def _sel(dst_sb, col, t):
                sel = spool.tile([P, t, P], bf16, tag="sel")
                import concourse.mybir as mybir
                nc.vector.scalar_tensor_tensor(
                    sel[:],
                    iota_sb[:, 0:t, :],
                    0.0,
                    dst_sb[:, col : col + t].unsqueeze(2).broadcast_to([P, t, P]),
                    mybir.AluOpType.add,
                    mybir.AluOpType.is_equal,
                )
                return sel

            def _emit_compute(ch, t_hi, a_tile, b_tile):
                for wi in range(cg):
                    w = ch * cg + wi
                    sel_l = _sel(dl_sb, w * t_lo, t_lo)
                    sel_h = _sel(dh_sb, int(ho_tiles[w]), t_hi)
                    psw = pwpool.tile([P, P], f32, tag="psw")
                    nmm = t_lo + t_hi
                    k = 0
                    for t in range(t_lo):
                        nc.tensor.matmul(
                            psw[:],
                            a_tile[:, wi * t_lo + t, :],
                            sel_l[:, t, :],
                            start=(k == 0),
                            stop=(k == nmm - 1),
                        )
                        k += 1
                    for t in range(t_hi):
                        nc.tensor.matmul(
                            psw[:],
                            b_tile[:, wi * t_hi + t, :],
                            sel_h[:, t, :],
                            start=(k == 0),
                            stop=(k == nmm - 1),
                        )
                        k += 1
                    # psw is aggT for this window: [dim, dst_local]
                    agg_t = gpool.tile([P, P], bf16, tag="agg")
                    nc.scalar.copy(agg_t[:], psw[:])
                    pso = popool.tile([P, P], f32, tag="pso")
                    nc.tensor.matmul(
                        pso[:], agg_t[:], w_sb[:], start=True, stop=True
                    )
                    o_sb = opool.tile([P, D], f32, tag="out")
                    nc.scalar.activation(
                        o_sb[:], pso[:], mybir.ActivationFunctionType.Relu
                    )
                    nc.sync.dma_start(
                        out=out.ap()[w * P : (w + 1) * P, :], in_=o_sb[:]
                    )

            if bench_reps == 1:
                body()
            else:
                # benchmarking only: repeat the whole body on-device so one
                # PJRT dispatch amortizes its ~90ms overhead over many runs
                with tc.For_i(0, bench_reps, 1):
                    body()

    nc.compile()
    _NC_CACHE[key] = nc
    return nc


def _balance(lo_cnt, hi_cnt, hi_caps, nw=NW, cap=P):
    """Greedy bin-packing of this core's dsts into nw windows of <=cap dsts,
    balancing lo load against 1024 and hi load against each window's cap."""
    nd = len(lo_cnt)
    order = np.argsort(-(lo_cnt + hi_cnt), kind="stable")
    wlo = np.zeros(nw)
    whi = np.zeros(nw)
    wn = np.zeros(nw, np.int64)
    win = np.empty(nd, np.int64)
    dloc = np.empty(nd, np.int64)
    lo_target = float(CAP_LO * P)
    hi_targets = hi_caps.astype(np.float64)
    for d in order:
        open_w = np.nonzero(wn < cap)[0]
        score = np.maximum(
            (wlo[open_w] + lo_cnt[d]) / lo_target,
            (whi[open_w] + hi_cnt[d]) / hi_targets[open_w],
        )
        w = open_w[np.argmin(score)]
        win[d] = w
        dloc[d] = wn[w]
        wlo[w] += lo_cnt[d]
        whi[w] += hi_cnt[d]
        wn[w] += 1
    return win, dloc, wlo, whi


def _grid(bucket, mask, order_vals_idx, order_vals_dloc, tiles_per_win,
          win_tile_off, nrows, nw=NW, n_cores=N_CORES):
    """Pack one src-half's edges into the per-core slot grid.

    bucket: per-edge (core * nw + window) id, mask: this half's edges.
    tiles_per_win/win_tile_off: per-window 128-slot tile counts + offsets.
    Returns idx16 [n_cores, 128, n/16] (int16, wrapped+replicated) and
    dloc [n_cores, 128, total_tiles] (f32, -1 for pad slots).
    """
    nb = n_cores * nw
    b = bucket[mask]
    order = np.argsort(b, kind="stable")
    b_sorted = b[order]
    cnts = np.bincount(b_sorted, minlength=nb)
    starts = np.concatenate([[0], np.cumsum(cnts)[:-1]])
    rank = np.arange(len(b_sorted)) - starts[b_sorted]
    total_tiles = int(tiles_per_win.sum())
    n = total_tiles * P
    # pad slots fetch the zeros row (row 0): repeated reads of one hot row
    # are DRAM row-buffer hits, cheaper than spreading pads randomly
    flat_idx = np.zeros((n_cores, n), dtype=np.int16)
    flat_dloc = np.full((n_cores, n), -1.0, dtype=np.float32)
    c = b_sorted // nw
    wloc = b_sorted % nw
    pos = win_tile_off[wloc] * P + rank
    flat_idx[c, pos] = order_vals_idx[mask][order]
    flat_dloc[c, pos] = order_vals_dloc[mask][order]
    idx16 = flat_idx.reshape(n_cores, n // 16, 16).transpose(0, 2, 1)
    idx16 = np.ascontiguousarray(np.tile(idx16, (1, 8, 1)))
    dl = np.ascontiguousarray(
        flat_dloc.reshape(n_cores, total_tiles, P).transpose(0, 2, 1)
    )
    return idx16, dl


def _bf16(a):
    import jax.numpy as jnp

    return np.asarray(jnp.asarray(np.asarray(a), dtype=jnp.bfloat16))


def kernel(x, edge_index, W):
    x = np.asarray(x, dtype=np.float32)
    edge_index = np.asarray(edge_index)
    W = np.asarray(W, dtype=np.float32)
    assert x.shape == (N_NODES, D) and W.shape == (D, D)

    src = edge_index[0].astype(np.int64)
    dst = edge_index[1].astype(np.int64)

    is_hi = src >= LO_CUT
    core = dst // DPC
    # per-core balanced window assignment
    lo_per_dst = np.bincount(dst[~is_hi], minlength=N_NODES)
    hi_per_dst = np.bincount(dst[is_hi], minlength=N_NODES)

    th = TH_DEFAULT
    t_hi_w, ho_tiles = _hi_tiles(th)
    hi_caps = t_hi_w * P

    w_of_dst = np.empty(N_NODES, np.int64)
    dloc_of_dst = np.empty(N_NODES, np.int64)
    max_lo = 0.0
    hi_ok = True
    for c in range(N_CORES):
        sl = slice(c * DPC, (c + 1) * DPC)
        win, dloc, wlo, whi = _balance(lo_per_dst[sl], hi_per_dst[sl], hi_caps)
        w_of_dst[sl] = win
        dloc_of_dst[sl] = dloc
        max_lo = max(max_lo, wlo.max())
        if np.any(whi > hi_caps):
            hi_ok = False
    if max_lo > CAP_LO * P or not hi_ok:
        # fallback: uniform generous capacities
        th = (5, 5, 5, 5, 5, 5, 5)
        t_hi_w, ho_tiles = _hi_tiles(th)
        hi_caps = t_hi_w * P
        t_lo = CAP_LO
        for c in range(N_CORES):
            sl = slice(c * DPC, (c + 1) * DPC)
            win, dloc, wlo, whi = _balance(lo_per_dst[sl], hi_per_dst[sl], hi_caps)
            w_of_dst[sl] = win
            dloc_of_dst[sl] = dloc
            t_lo = max(t_lo, int(np.ceil(wlo.max() / P)))
            t_hi_w = np.maximum(t_hi_w, int(np.ceil(whi.max() / P)))
        th = tuple(int(t_hi_w[c * CG]) for c in range(NCH))
        t_hi_w, ho_tiles = _hi_tiles(th)
    else:
        t_lo = CAP_LO

    lo_tiles_w = np.full(NW, t_lo, np.int64)
    lo_off = np.concatenate([[0], np.cumsum(lo_tiles_w)[:-1]])

    w_all = w_of_dst[dst]
    dloc_all = dloc_of_dst[dst].astype(np.float32)
    bucket = core * NW + w_all

    idx_val_lo = (src + 1).astype(np.int16, casting="unsafe")
    idx_val_hi = (src - LO_CUT + 1).astype(np.int16, casting="unsafe")
    idx16_lo, dloc_lo = _grid(bucket, ~is_hi, idx_val_lo, dloc_all,
                              lo_tiles_w, lo_off, LO_ROWS - 1)
    idx16_hi, dloc_hi = _grid(bucket, is_hi, idx_val_hi, dloc_all,
                              t_hi_w, ho_tiles, HI_ROWS - 1)

    tbl_lo = np.zeros((LO_ROWS, D), np.float32)
    tbl_lo[1:] = x[:LO_CUT]
    tbl_hi = np.zeros((HI_ROWS, D), np.float32)
    tbl_hi[1:] = x[LO_CUT:]
    tmax = max(t_lo, max(th))
    iota = np.tile(np.arange(P, dtype=np.float32), (P, tmax))
    iota = np.ascontiguousarray(iota)

    nc = _build_nc(t_lo, th)

    tbl_lo16 = _bf16(tbl_lo)
    tbl_hi16 = _bf16(tbl_hi)
    w16 = _bf16(W)
    iota16 = _bf16(iota)
    in_maps = []
    for c in range(N_CORES):
        in_maps.append(
            {
                "tbl_lo": tbl_lo16,
                "tbl_hi": tbl_hi16,
                "idx_lo": idx16_lo[c],
                "idx_hi": idx16_hi[c],
                "dloc_lo": _bf16(dloc_lo[c]),
                "dloc_hi": _bf16(dloc_hi[c]),
                "wmat": w16,
                "iota": iota16,
                # only read by the parts="compute" ablation variant
                "dummy_a": _bf16(np.full((P, CG * t_lo, P), 0.5, np.float32)),
                "dummy_b": _bf16(np.full((P, CG * max(th), P), 0.5, np.float32)),
            }
        )

    from concourse.bass_utils import run_bass_kernel_spmd

    res = run_bass_kernel_spmd(nc, in_maps, core_ids=list(range(N_CORES)))
    # stashed so a test harness can re-run / re-time this invocation
    global _LAST_RUN, _LAST_CAPS
    _LAST_RUN = (nc, in_maps)
    _LAST_CAPS = (t_lo, th)
    outs = np.stack([res.results[c]["out"] for c in range(N_CORES)])  # [8, NW*P, D]

    # unpermute: dst n lives at core n//DPC, row w_of_dst[n]*P + dloc_of_dst[n]
    full = outs[np.arange(N_NODES) // DPC, w_of_dst * P + dloc_of_dst]
    return np.ascontiguousarray(full)


_LAST_RUN = None
_LAST_CAPS = None


# revision 18
# speedup vs baseline: 1.8855x; 1.8855x over previous
"""GNN message passing on 8 Trainium2 NeuronCores.

Reference computation:
    h = x @ W                       # [N, D]
    msg = h[src]                    # [E, D]
    out = relu(segment_sum(msg, dst, N))

Key identity: segment_sum(x[src]) @ W == segment_sum(x[src] @ W), so we
aggregate raw x rows and apply the small 128x128 matmul once per output
window at the end.

Distribution: shard edges by DESTINATION block (6250 dsts per core), so
outputs are disjoint and no all-reduce is needed. Each core:
  1. dma_gather's bf16 x rows for its edges from a replicated node table
     in HBM (split lo/hi so gather indices fit in int16),
  2. scatters each 128-slot tile into per-window PSUM accumulators via
     TensorE matmuls against one-hot selection matrices built on-device
     with one DVE compare (iota == dloc) per window,
  3. applies the final @W matmul + relu per window and DMAs out fp32.

The dma_gather cost is per-index (gpsimd ucode descriptor generation +
SWDGE ring drain).  Two things matter: splitting each chunk's gathers
into pieces rotated across all 4 SWDGE queues (different DSP pairs +
rings pipeline; a single queue serializes generation against ring
drain, ~2.2x slower), and minimizing slot count.  For the latter the
host BALANCES the dst->window assignment per core
(greedy bin packing on per-dst lo/hi edge counts): every window fits
t_lo=8 lo tiles, and hi capacity alternates by chunk (chunks 0-3 get 5
hi tiles per window, chunks 4-6 get 4) with hi-heavy dsts packed into
the roomier windows.  The host unpermutes the output rows at the end.
"""

import numpy as np

P = 128
D = 128
N_NODES = 50000
N_CORES = 8
NW = 49                 # 128-dst windows per core
DPC = N_NODES // N_CORES        # real dsts per core = 6250
CG = 7                  # windows per gather chunk (NW % CG == 0)
NCH = NW // CG
LO_CUT = 32767          # src < LO_CUT -> lo table (idx = src+1 <= 32767)
LO_ROWS = LO_CUT + 1
HI_ROWS = N_NODES - LO_CUT + 1
CAP_LO = 8              # lo tiles per window
TH_DEFAULT = (5, 5, 5, 5, 4, 4, 4)   # hi tiles per window, by chunk

_NC_CACHE = {}


def _hi_tiles(th):
    """Per-window hi tile counts and tile-offset prefix for pattern th."""
    t_w = np.repeat(np.asarray(th, np.int64), CG)          # [NW]
    off = np.concatenate([[0], np.cumsum(t_w)[:-1]])       # tile offset per window
    return t_w, off


def _build_nc(t_lo, th, nw=NW, cg=CG, lo_rows=LO_ROWS, hi_rows=HI_ROWS,
              bench_reps=1, parts="all", dma_scratch=16384, n_queues=4,
              qrot=False, qsplit=4):
    key = (t_lo, tuple(th), nw, cg, lo_rows, hi_rows, bench_reps, parts,
           dma_scratch, n_queues, qrot, qsplit, "v3")
    if key in _NC_CACHE:
        return _NC_CACHE[key]

    import concourse.bacc as bacc
    import concourse.mybir as mybir
    import concourse.tile as tile
    from concourse import library_config

    assert nw % cg == 0
    nch = nw // cg
    t_hi_w, ho_tiles = _hi_tiles(th)
    n_lo = nw * t_lo * P
    hi_tiles_total = int(t_hi_w.sum())
    n_hi = hi_tiles_total * P
    th_max = max(th)
    tmax = max(t_lo, th_max)

    nc = bacc.Bacc(
        "TRN2", target_bir_lowering=False, debug=False, num_swdge_queues=n_queues,
        dynamic_dma_scratch_size=dma_scratch,
    )
    f32 = mybir.dt.float32
    bf16 = mybir.dt.bfloat16
    tbl_lo = nc.dram_tensor("tbl_lo", [lo_rows, D], bf16, kind="ExternalInput")
    tbl_hi = nc.dram_tensor("tbl_hi", [hi_rows, D], bf16, kind="ExternalInput")
    idx_lo = nc.dram_tensor(
        "idx_lo", [P, n_lo // 16], mybir.dt.int16, kind="ExternalInput"
    )
    idx_hi = nc.dram_tensor(
        "idx_hi", [P, n_hi // 16], mybir.dt.int16, kind="ExternalInput"
    )
    dloc_lo = nc.dram_tensor("dloc_lo", [P, nw * t_lo], bf16, kind="ExternalInput")
    dloc_hi = nc.dram_tensor("dloc_hi", [P, hi_tiles_total], bf16,
                             kind="ExternalInput")
    wmat = nc.dram_tensor("wmat", [D, D], bf16, kind="ExternalInput")
    iota = nc.dram_tensor("iota", [P, tmax * P], bf16, kind="ExternalInput")
    out = nc.dram_tensor("out", [nw * P, D], f32, kind="ExternalOutput")
    if parts == "compute":
        dummy_a = nc.dram_tensor(
            "dummy_a", [P, cg * t_lo, P], bf16, kind="ExternalInput"
        )
        dummy_b = nc.dram_tensor(
            "dummy_b", [P, cg * th_max, P], bf16, kind="ExternalInput"
        )

    with tile.TileContext(nc) as tc:
        nc.gpsimd.load_library(library_config.mlp)
        with (
            tc.tile_pool(name="const", bufs=1) as cpool,
            tc.tile_pool(name="msga", bufs=3) as apool,
            tc.tile_pool(name="msgb", bufs=3) as bpool,
            tc.tile_pool(name="sel", bufs=6) as spool,
            tc.tile_pool(name="agg", bufs=4) as gpool,
            tc.tile_pool(name="outp", bufs=4) as opool,
            tc.tile_pool(name="psw", bufs=4, space="PSUM") as pwpool,
            tc.tile_pool(name="pso", bufs=2, space="PSUM") as popool,
        ):
            w_sb = cpool.tile([D, D], bf16, tag="w")
            nc.sync.dma_start(out=w_sb[:], in_=wmat.ap())
            iota_sb = cpool.tile([P, tmax, P], bf16, tag="iota")
            nc.sync.dma_start(
                out=iota_sb[:], in_=iota.ap().rearrange("p (t j) -> p t j", t=tmax)
            )
            il_sb = cpool.tile([P, n_lo // 16], mybir.dt.int16, tag="il")
            nc.sync.dma_start(out=il_sb[:], in_=idx_lo.ap())
            ih_sb = cpool.tile([P, n_hi // 16], mybir.dt.int16, tag="ih")
            nc.sync.dma_start(out=ih_sb[:], in_=idx_hi.ap())
            dl_sb = cpool.tile([P, nw * t_lo], bf16, tag="dl")
            nc.sync.dma_start(out=dl_sb[:], in_=dloc_lo.ap())
            dh_sb = cpool.tile([P, hi_tiles_total], bf16, tag="dh")
            nc.sync.dma_start(out=dh_sb[:], in_=dloc_hi.ap())

            def body():
              for ch in range(nch):
                t_hi = th[ch]
                a_tile = apool.tile([P, cg * t_lo, D], bf16, tag="msga")
                b_tile = bpool.tile([P, cg * t_hi, D], bf16, tag="msgb")
                if parts in ("all", "gather"):
                    _emit_gathers(ch, t_hi, a_tile, b_tile)
                if parts == "compute":
                    nc.sync.dma_start(out=a_tile[:], in_=dummy_a.ap())
                    nc.sync.dma_start(
                        out=b_tile[:], in_=dummy_b.ap()[:, 0 : cg * t_hi, :]
                    )
                if parts in ("all", "compute"):
                    _emit_compute(ch, t_hi, a_tile, b_tile)
                if parts == "gather":
                    psw = pwpool.tile([P, P], f32, tag="psw")
                    nc.tensor.matmul(
                        psw[:], a_tile[:, 0, :], b_tile[:, 0, :],
                        start=True, stop=True,
                    )
                    o_sb = opool.tile([P, D], f32, tag="out")
                    nc.scalar.copy(o_sb[:], psw[:])
                    nc.sync.dma_start(out=out.ap()[0:P, :], in_=o_sb[:])

            def _gather_piece(tile_sb, tbl, idx_sb, tile0, ntiles, q):
                nc.gpsimd.dma_gather(
                    tile_sb,
                    tbl.ap(),
                    idx_sb[:, tile0 * 8 : (tile0 + ntiles) * 8],
                    ntiles * P,
                    ntiles * P,
                    D,
                    queue_num=q,
                    single_packet=False,
                )

            def _emit_gathers(ch, t_hi, a_tile, b_tile):
                q_lo = (2 * (ch % 2)) if qrot else 0
                q_hi = (1 + 2 * (ch % 2)) if qrot else (1 % n_queues)
                nlo_t = cg * t_lo
                nhi_t = cg * t_hi
                hi_tile_base = int(ho_tiles[ch * cg])
                if qsplit == 1:
                    _gather_piece(a_tile[:], tbl_lo, il_sb, ch * nlo_t, nlo_t, q_lo)
                    _gather_piece(b_tile[:], tbl_hi, ih_sb, hi_tile_base, nhi_t, q_hi)
                else:
                    lo_b = [nlo_t * j // qsplit for j in range(qsplit + 1)]
                    hi_b = [nhi_t * j // qsplit for j in range(qsplit + 1)]
                    q = 0
                    for j in range(qsplit):
                        _gather_piece(
                            a_tile[:, lo_b[j] : lo_b[j + 1], :], tbl_lo, il_sb,
                            ch * nlo_t + lo_b[j], lo_b[j + 1] - lo_b[j], q % 4)
                        q += 1
                        _gather_piece(
                            b_tile[:, hi_b[j] : hi_b[j + 1], :], tbl_hi, ih_sb,
                            hi_tile_base + hi_b[j], hi_b[j + 1] - hi_b[j], q % 4)
                        q += 1

            def _sel(dst_sb, col, t):
                sel = spool.tile([P, t, P], bf16, tag="sel")
                import concourse.mybir as mybir
                nc.vector.scalar_tensor_tensor(
                    sel[:],
                    iota_sb[:, 0:t, :],
                    0.0,
                    dst_sb[:, col : col + t].unsqueeze(2).broadcast_to([P, t, P]),
                    mybir.AluOpType.add,
                    mybir.AluOpType.is_equal,
                )
                return sel

            def _emit_compute(ch, t_hi, a_tile, b_tile):
                for wi in range(cg):
                    w = ch * cg + wi
                    sel_l = _sel(dl_sb, w * t_lo, t_lo)
                    sel_h = _sel(dh_sb, int(ho_tiles[w]), t_hi)
                    psw = pwpool.tile([P, P], f32, tag="psw")
                    nmm = t_lo + t_hi
                    k = 0
                    for t in range(t_lo):
                        nc.tensor.matmul(
                            psw[:],
                            a_tile[:, wi * t_lo + t, :],
                            sel_l[:, t, :],
                            start=(k == 0),
                            stop=(k == nmm - 1),
                        )
                        k += 1
                    for t in range(t_hi):
                        nc.tensor.matmul(
                            psw[:],
                            b_tile[:, wi * t_hi + t, :],
                            sel_h[:, t, :],
                            start=(k == 0),
                            stop=(k == nmm - 1),
                        )
                        k += 1
                    # psw is aggT for this window: [dim, dst_local]
                    agg_t = gpool.tile([P, P], bf16, tag="agg")
                    nc.scalar.copy(agg_t[:], psw[:])
                    pso = popool.tile([P, P], f32, tag="pso")
                    nc.tensor.matmul(
                        pso[:], agg_t[:], w_sb[:], start=True, stop=True
                    )
                    o_sb = opool.tile([P, D], f32, tag="out")
                    nc.scalar.activation(
                        o_sb[:], pso[:], mybir.ActivationFunctionType.Relu
                    )
                    nc.sync.dma_start(
                        out=out.ap()[w * P : (w + 1) * P, :], in_=o_sb[:]
                    )

            if bench_reps == 1:
                body()
            else:
                # benchmarking only: repeat the whole body on-device so one
                # PJRT dispatch amortizes its ~90ms overhead over many runs
                with tc.For_i(0, bench_reps, 1):
                    body()

    nc.compile()
    _NC_CACHE[key] = nc
    return nc


def _balance(lo_cnt, hi_cnt, hi_caps, nw=NW, cap=P):
    """Greedy bin-packing of this core's dsts into nw windows of <=cap dsts,
    balancing lo load against 1024 and hi load against each window's cap."""
    nd = len(lo_cnt)
    order = np.argsort(-(lo_cnt + hi_cnt), kind="stable")
    wlo = np.zeros(nw)
    whi = np.zeros(nw)
    wn = np.zeros(nw, np.int64)
    win = np.empty(nd, np.int64)
    dloc = np.empty(nd, np.int64)
    lo_target = float(CAP_LO * P)
    hi_targets = hi_caps.astype(np.float64)
    for d in order:
        open_w = np.nonzero(wn < cap)[0]
        score = np.maximum(
            (wlo[open_w] + lo_cnt[d]) / lo_target,
            (whi[open_w] + hi_cnt[d]) / hi_targets[open_w],
        )
        w = open_w[np.argmin(score)]
        win[d] = w
        dloc[d] = wn[w]
        wlo[w] += lo_cnt[d]
        whi[w] += hi_cnt[d]
        wn[w] += 1
    return win, dloc, wlo, whi


def _grid(bucket, mask, order_vals_idx, order_vals_dloc, tiles_per_win,
          win_tile_off, nrows, nw=NW, n_cores=N_CORES):
    """Pack one src-half's edges into the per-core slot grid.

    bucket: per-edge (core * nw + window) id, mask: this half's edges.
    tiles_per_win/win_tile_off: per-window 128-slot tile counts + offsets.
    Returns idx16 [n_cores, 128, n/16] (int16, wrapped+replicated) and
    dloc [n_cores, 128, total_tiles] (f32, -1 for pad slots).
    """
    nb = n_cores * nw
    b = bucket[mask]
    order = np.argsort(b, kind="stable")
    b_sorted = b[order]
    cnts = np.bincount(b_sorted, minlength=nb)
    starts = np.concatenate([[0], np.cumsum(cnts)[:-1]])
    rank = np.arange(len(b_sorted)) - starts[b_sorted]
    total_tiles = int(tiles_per_win.sum())
    n = total_tiles * P
    # pad slots fetch the zeros row (row 0): repeated reads of one hot row
    # are DRAM row-buffer hits, cheaper than spreading pads randomly
    flat_idx = np.zeros((n_cores, n), dtype=np.int16)
    flat_dloc = np.full((n_cores, n), -1.0, dtype=np.float32)
    c = b_sorted // nw
    wloc = b_sorted % nw
    pos = win_tile_off[wloc] * P + rank
    flat_idx[c, pos] = order_vals_idx[mask][order]
    flat_dloc[c, pos] = order_vals_dloc[mask][order]
    idx16 = flat_idx.reshape(n_cores, n // 16, 16).transpose(0, 2, 1)
    idx16 = np.ascontiguousarray(np.tile(idx16, (1, 8, 1)))
    dl = np.ascontiguousarray(
        flat_dloc.reshape(n_cores, total_tiles, P).transpose(0, 2, 1)
    )
    return idx16, dl


def _bf16(a):
    import jax.numpy as jnp

    return np.asarray(jnp.asarray(np.asarray(a), dtype=jnp.bfloat16))


def kernel(x, edge_index, W):
    x = np.asarray(x, dtype=np.float32)
    edge_index = np.asarray(edge_index)
    W = np.asarray(W, dtype=np.float32)
    assert x.shape == (N_NODES, D) and W.shape == (D, D)

    src = edge_index[0].astype(np.int64)
    dst = edge_index[1].astype(np.int64)

    is_hi = src >= LO_CUT
    core = dst // DPC
    # per-core balanced window assignment
    lo_per_dst = np.bincount(dst[~is_hi], minlength=N_NODES)
    hi_per_dst = np.bincount(dst[is_hi], minlength=N_NODES)

    th = TH_DEFAULT
    t_hi_w, ho_tiles = _hi_tiles(th)
    hi_caps = t_hi_w * P

    w_of_dst = np.empty(N_NODES, np.int64)
    dloc_of_dst = np.empty(N_NODES, np.int64)
    max_lo = 0.0
    hi_ok = True
    for c in range(N_CORES):
        sl = slice(c * DPC, (c + 1) * DPC)
        win, dloc, wlo, whi = _balance(lo_per_dst[sl], hi_per_dst[sl], hi_caps)
        w_of_dst[sl] = win
        dloc_of_dst[sl] = dloc
        max_lo = max(max_lo, wlo.max())
        if np.any(whi > hi_caps):
            hi_ok = False
    if max_lo > CAP_LO * P or not hi_ok:
        # fallback: uniform generous capacities
        th = (5, 5, 5, 5, 5, 5, 5)
        t_hi_w, ho_tiles = _hi_tiles(th)
        hi_caps = t_hi_w * P
        t_lo = CAP_LO
        for c in range(N_CORES):
            sl = slice(c * DPC, (c + 1) * DPC)
            win, dloc, wlo, whi = _balance(lo_per_dst[sl], hi_per_dst[sl], hi_caps)
            w_of_dst[sl] = win
            dloc_of_dst[sl] = dloc
            t_lo = max(t_lo, int(np.ceil(wlo.max() / P)))
            t_hi_w = np.maximum(t_hi_w, int(np.ceil(whi.max() / P)))
        th = tuple(int(t_hi_w[c * CG]) for c in range(NCH))
        t_hi_w, ho_tiles = _hi_tiles(th)
    else:
        t_lo = CAP_LO

    lo_tiles_w = np.full(NW, t_lo, np.int64)
    lo_off = np.concatenate([[0], np.cumsum(lo_tiles_w)[:-1]])

    w_all = w_of_dst[dst]
    dloc_all = dloc_of_dst[dst].astype(np.float32)
    bucket = core * NW + w_all

    idx_val_lo = (src + 1).astype(np.int16, casting="unsafe")
    idx_val_hi = (src - LO_CUT + 1).astype(np.int16, casting="unsafe")
    idx16_lo, dloc_lo = _grid(bucket, ~is_hi, idx_val_lo, dloc_all,
                              lo_tiles_w, lo_off, LO_ROWS - 1)
    idx16_hi, dloc_hi = _grid(bucket, is_hi, idx_val_hi, dloc_all,
                              t_hi_w, ho_tiles, HI_ROWS - 1)

    tbl_lo = np.zeros((LO_ROWS, D), np.float32)
    tbl_lo[1:] = x[:LO_CUT]
    tbl_hi = np.zeros((HI_ROWS, D), np.float32)
    tbl_hi[1:] = x[LO_CUT:]
    tmax = max(t_lo, max(th))
    iota = np.tile(np.arange(P, dtype=np.float32), (P, tmax))
    iota = np.ascontiguousarray(iota)

    nc = _build_nc(t_lo, th)

    tbl_lo16 = _bf16(tbl_lo)
    tbl_hi16 = _bf16(tbl_hi)
    w16 = _bf16(W)
    iota16 = _bf16(iota)
    in_maps = []
    for c in range(N_CORES):
        in_maps.append(
            {
                "tbl_lo": tbl_lo16,
                "tbl_hi": tbl_hi16,
                "idx_lo": idx16_lo[c],
                "idx_hi": idx16_hi[c],
                "dloc_lo": _bf16(dloc_lo[c]),
                "dloc_hi": _bf16(dloc_hi[c]),
                "wmat": w16,
                "iota": iota16,
                # only read by the parts="compute" ablation variant
                "dummy_a": _bf16(np.full((P, CG * t_lo, P), 0.5, np.float32)),
                "dummy_b": _bf16(np.full((P, CG * max(th), P), 0.5, np.float32)),
            }
        )

    from concourse.bass_utils import run_bass_kernel_spmd

    res = run_bass_kernel_spmd(nc, in_maps, core_ids=list(range(N_CORES)))
    # stashed so a test harness can re-run / re-time this invocation
    global _LAST_RUN, _LAST_CAPS
    _LAST_RUN = (nc, in_maps)
    _LAST_CAPS = (t_lo, th)
    outs = np.stack([res.results[c]["out"] for c in range(N_CORES)])  # [8, NW*P, D]

    # unpermute: dst n lives at core n//DPC, row w_of_dst[n]*P + dloc_of_dst[n]
    full = outs[np.arange(N_NODES) // DPC, w_of_dst * P + dloc_of_dst]
    return np.ascontiguousarray(full)


_LAST_RUN = None
_LAST_CAPS = None


# revision 19
# speedup vs baseline: 1.9425x; 1.0302x over previous
"""GNN message passing on 8 Trainium2 NeuronCores.

Reference computation:
    h = x @ W                       # [N, D]
    msg = h[src]                    # [E, D]
    out = relu(segment_sum(msg, dst, N))

Key identity: segment_sum(x[src]) @ W == segment_sum(x[src] @ W), so we
aggregate raw x rows and apply the small 128x128 matmul once per output
window at the end.

Distribution: shard edges by DESTINATION block (6250 dsts per core), so
outputs are disjoint and no all-reduce is needed. Each core:
  1. dma_gather's bf16 x rows for its edges from a replicated node table
     in HBM (split lo/hi so gather indices fit in int16),
  2. scatters each 128-slot tile into per-window PSUM accumulators via
     TensorE matmuls against one-hot selection matrices built on-device
     with one DVE compare (iota == dloc) per window,
  3. applies the final @W matmul + relu per window and DMAs out fp32.

The dma_gather cost is per-index (gpsimd ucode descriptor generation +
SWDGE ring drain).  Two things matter: splitting each chunk's gathers
into pieces rotated across all 4 SWDGE queues (different DSP pairs +
rings pipeline; a single queue serializes generation against ring
drain, ~2.2x slower), and minimizing slot count.  For the latter the
host BALANCES the dst->window assignment per core
(greedy bin packing on per-dst lo/hi edge counts): every window fits
t_lo=8 lo tiles, and hi capacity varies by chunk (chunks 0-1 get 5 hi
tiles per window, chunks 2-6 get 4) with hi-heavy dsts packed into the
roomier windows.  The host unpermutes the output rows at the end.
"""

import numpy as np

P = 128
D = 128
N_NODES = 50000
N_CORES = 8
NW = 49                 # 128-dst windows per core
DPC = N_NODES // N_CORES        # real dsts per core = 6250
CG = 7                  # windows per gather chunk (NW % CG == 0)
NCH = NW // CG
LO_CUT = 32767          # src < LO_CUT -> lo table (idx = src+1 <= 32767)
LO_ROWS = LO_CUT + 1
HI_ROWS = N_NODES - LO_CUT + 1
CAP_LO = 8              # lo tiles per window
TH_DEFAULT = (5, 5, 4, 4, 4, 4, 4)   # hi tiles per window, by chunk

_NC_CACHE = {}


def _hi_tiles(th):
    """Per-window hi tile counts and tile-offset prefix for pattern th."""
    t_w = np.repeat(np.asarray(th, np.int64), CG)          # [NW]
    off = np.concatenate([[0], np.cumsum(t_w)[:-1]])       # tile offset per window
    return t_w, off


def _build_nc(t_lo, th, nw=NW, cg=CG, lo_rows=LO_ROWS, hi_rows=HI_ROWS,
              bench_reps=1, parts="all", dma_scratch=16384, n_queues=4,
              qrot=False, qsplit=4):
    key = (t_lo, tuple(th), nw, cg, lo_rows, hi_rows, bench_reps, parts,
           dma_scratch, n_queues, qrot, qsplit, "v3")
    if key in _NC_CACHE:
        return _NC_CACHE[key]

    import concourse.bacc as bacc
    import concourse.mybir as mybir
    import concourse.tile as tile
    from concourse import library_config

    assert nw % cg == 0
    nch = nw // cg
    t_hi_w, ho_tiles = _hi_tiles(th)
    n_lo = nw * t_lo * P
    hi_tiles_total = int(t_hi_w.sum())
    n_hi = hi_tiles_total * P
    th_max = max(th)
    tmax = max(t_lo, th_max)

    nc = bacc.Bacc(
        "TRN2", target_bir_lowering=False, debug=False, num_swdge_queues=n_queues,
        dynamic_dma_scratch_size=dma_scratch,
    )
    f32 = mybir.dt.float32
    bf16 = mybir.dt.bfloat16
    tbl_lo = nc.dram_tensor("tbl_lo", [lo_rows, D], bf16, kind="ExternalInput")
    tbl_hi = nc.dram_tensor("tbl_hi", [hi_rows, D], bf16, kind="ExternalInput")
    idx_lo = nc.dram_tensor(
        "idx_lo", [P, n_lo // 16], mybir.dt.int16, kind="ExternalInput"
    )
    idx_hi = nc.dram_tensor(
        "idx_hi", [P, n_hi // 16], mybir.dt.int16, kind="ExternalInput"
    )
    dloc_lo = nc.dram_tensor("dloc_lo", [P, nw * t_lo], bf16, kind="ExternalInput")
    dloc_hi = nc.dram_tensor("dloc_hi", [P, hi_tiles_total], bf16,
                             kind="ExternalInput")
    wmat = nc.dram_tensor("wmat", [D, D], bf16, kind="ExternalInput")
    iota = nc.dram_tensor("iota", [P, tmax * P], bf16, kind="ExternalInput")
    out = nc.dram_tensor("out", [nw * P, D], f32, kind="ExternalOutput")
    if parts == "compute":
        dummy_a = nc.dram_tensor(
            "dummy_a", [P, cg * t_lo, P], bf16, kind="ExternalInput"
        )
        dummy_b = nc.dram_tensor(
            "dummy_b", [P, cg * th_max, P], bf16, kind="ExternalInput"
        )

    with tile.TileContext(nc) as tc:
        nc.gpsimd.load_library(library_config.mlp)
        with (
            tc.tile_pool(name="const", bufs=1) as cpool,
            tc.tile_pool(name="msga", bufs=3) as apool,
            tc.tile_pool(name="msgb", bufs=3) as bpool,
            tc.tile_pool(name="sel", bufs=6) as spool,
            tc.tile_pool(name="agg", bufs=4) as gpool,
            tc.tile_pool(name="outp", bufs=4) as opool,
            tc.tile_pool(name="psw", bufs=4, space="PSUM") as pwpool,
            tc.tile_pool(name="pso", bufs=2, space="PSUM") as popool,
        ):
            w_sb = cpool.tile([D, D], bf16, tag="w")
            nc.sync.dma_start(out=w_sb[:], in_=wmat.ap())
            iota_sb = cpool.tile([P, tmax, P], bf16, tag="iota")
            nc.sync.dma_start(
                out=iota_sb[:], in_=iota.ap().rearrange("p (t j) -> p t j", t=tmax)
            )
            il_sb = cpool.tile([P, n_lo // 16], mybir.dt.int16, tag="il")
            nc.sync.dma_start(out=il_sb[:], in_=idx_lo.ap())
            ih_sb = cpool.tile([P, n_hi // 16], mybir.dt.int16, tag="ih")
            nc.sync.dma_start(out=ih_sb[:], in_=idx_hi.ap())
            dl_sb = cpool.tile([P, nw * t_lo], bf16, tag="dl")
            nc.sync.dma_start(out=dl_sb[:], in_=dloc_lo.ap())
            dh_sb = cpool.tile([P, hi_tiles_total], bf16, tag="dh")
            nc.sync.dma_start(out=dh_sb[:], in_=dloc_hi.ap())

            def body():
              for ch in range(nch):
                t_hi = th[ch]
                a_tile = apool.tile([P, cg * t_lo, D], bf16, tag="msga")
                b_tile = bpool.tile([P, cg * t_hi, D], bf16, tag="msgb")
                if parts in ("all", "gather"):
                    _emit_gathers(ch, t_hi, a_tile, b_tile)
                if parts == "compute":
                    nc.sync.dma_start(out=a_tile[:], in_=dummy_a.ap())
                    nc.sync.dma_start(
                        out=b_tile[:], in_=dummy_b.ap()[:, 0 : cg * t_hi, :]
                    )
                if parts in ("all", "compute"):
                    _emit_compute(ch, t_hi, a_tile, b_tile)
                if parts == "gather":
                    psw = pwpool.tile([P, P], f32, tag="psw")
                    nc.tensor.matmul(
                        psw[:], a_tile[:, 0, :], b_tile[:, 0, :],
                        start=True, stop=True,
                    )
                    o_sb = opool.tile([P, D], f32, tag="out")
                    nc.scalar.copy(o_sb[:], psw[:])
                    nc.sync.dma_start(out=out.ap()[0:P, :], in_=o_sb[:])

            def _gather_piece(tile_sb, tbl, idx_sb, tile0, ntiles, q):
                nc.gpsimd.dma_gather(
                    tile_sb,
                    tbl.ap(),
                    idx_sb[:, tile0 * 8 : (tile0 + ntiles) * 8],
                    ntiles * P,
                    ntiles * P,
                    D,
                    queue_num=q,
                    single_packet=False,
                )

            def _emit_gathers(ch, t_hi, a_tile, b_tile):
                q_lo = (2 * (ch % 2)) if qrot else 0
                q_hi = (1 + 2 * (ch % 2)) if qrot else (1 % n_queues)
                nlo_t = cg * t_lo
                nhi_t = cg * t_hi
                hi_tile_base = int(ho_tiles[ch * cg])
                if qsplit == 1:
                    _gather_piece(a_tile[:], tbl_lo, il_sb, ch * nlo_t, nlo_t, q_lo)
                    _gather_piece(b_tile[:], tbl_hi, ih_sb, hi_tile_base, nhi_t, q_hi)
                else:
                    lo_b = [nlo_t * j // qsplit for j in range(qsplit + 1)]
                    hi_b = [nhi_t * j // qsplit for j in range(qsplit + 1)]
                    q = 0
                    for j in range(qsplit):
                        _gather_piece(
                            a_tile[:, lo_b[j] : lo_b[j + 1], :], tbl_lo, il_sb,
                            ch * nlo_t + lo_b[j], lo_b[j + 1] - lo_b[j], q % 4)
                        q += 1
                        _gather_piece(
                            b_tile[:, hi_b[j] : hi_b[j + 1], :], tbl_hi, ih_sb,
                            hi_tile_base + hi_b[j], hi_b[j + 1] - hi_b[j], q % 4)
                        q += 1

            def _sel(dst_sb, col, t):
                sel = spool.tile([P, t, P], bf16, tag="sel")
                import concourse.mybir as mybir
                nc.vector.scalar_tensor_tensor(
                    sel[:],
                    iota_sb[:, 0:t, :],
                    0.0,
                    dst_sb[:, col : col + t].unsqueeze(2).broadcast_to([P, t, P]),
                    mybir.AluOpType.add,
                    mybir.AluOpType.is_equal,
                )
                return sel

            def _emit_compute(ch, t_hi, a_tile, b_tile):
                for wi in range(cg):
                    w = ch * cg + wi
                    sel_l = _sel(dl_sb, w * t_lo, t_lo)
                    sel_h = _sel(dh_sb, int(ho_tiles[w]), t_hi)
                    psw = pwpool.tile([P, P], f32, tag="psw")
                    nmm = t_lo + t_hi
                    k = 0
                    for t in range(t_lo):
                        nc.tensor.matmul(
                            psw[:],
                            a_tile[:, wi * t_lo + t, :],
                            sel_l[:, t, :],
                            start=(k == 0),
                            stop=(k == nmm - 1),
                        )
                        k += 1
                    for t in range(t_hi):
                        nc.tensor.matmul(
                            psw[:],
                            b_tile[:, wi * t_hi + t, :],
                            sel_h[:, t, :],
                            start=(k == 0),
                            stop=(k == nmm - 1),
                        )
                        k += 1
                    # psw is aggT for this window: [dim, dst_local]
                    agg_t = gpool.tile([P, P], bf16, tag="agg")
                    nc.scalar.copy(agg_t[:], psw[:])
                    pso = popool.tile([P, P], f32, tag="pso")
                    nc.tensor.matmul(
                        pso[:], agg_t[:], w_sb[:], start=True, stop=True
                    )
                    o_sb = opool.tile([P, D], f32, tag="out")
                    nc.scalar.activation(
                        o_sb[:], pso[:], mybir.ActivationFunctionType.Relu
                    )
                    nc.sync.dma_start(
                        out=out.ap()[w * P : (w + 1) * P, :], in_=o_sb[:]
                    )

            if bench_reps == 1:
                body()
            else:
                # benchmarking only: repeat the whole body on-device so one
                # PJRT dispatch amortizes its ~90ms overhead over many runs
                with tc.For_i(0, bench_reps, 1):
                    body()

    nc.compile()
    _NC_CACHE[key] = nc
    return nc


def _balance(lo_cnt, hi_cnt, hi_caps, nw=NW, cap=P):
    """Greedy bin-packing of this core's dsts into nw windows of <=cap dsts,
    balancing lo load against 1024 and hi load against each window's cap."""
    nd = len(lo_cnt)
    order = np.argsort(-(lo_cnt + hi_cnt), kind="stable")
    wlo = np.zeros(nw)
    whi = np.zeros(nw)
    wn = np.zeros(nw, np.int64)
    win = np.empty(nd, np.int64)
    dloc = np.empty(nd, np.int64)
    lo_target = float(CAP_LO * P)
    hi_targets = hi_caps.astype(np.float64)
    for d in order:
        open_w = np.nonzero(wn < cap)[0]
        score = np.maximum(
            (wlo[open_w] + lo_cnt[d]) / lo_target,
            (whi[open_w] + hi_cnt[d]) / hi_targets[open_w],
        )
        w = open_w[np.argmin(score)]
        win[d] = w
        dloc[d] = wn[w]
        wlo[w] += lo_cnt[d]
        whi[w] += hi_cnt[d]
        wn[w] += 1
    return win, dloc, wlo, whi


def _grid(bucket, mask, order_vals_idx, order_vals_dloc, tiles_per_win,
          win_tile_off, nrows, nw=NW, n_cores=N_CORES):
    """Pack one src-half's edges into the per-core slot grid.

    bucket: per-edge (core * nw + window) id, mask: this half's edges.
    tiles_per_win/win_tile_off: per-window 128-slot tile counts + offsets.
    Returns idx16 [n_cores, 128, n/16] (int16, wrapped+replicated) and
    dloc [n_cores, 128, total_tiles] (f32, -1 for pad slots).
    """
    nb = n_cores * nw
    b = bucket[mask]
    order = np.argsort(b, kind="stable")
    b_sorted = b[order]
    cnts = np.bincount(b_sorted, minlength=nb)
    starts = np.concatenate([[0], np.cumsum(cnts)[:-1]])
    rank = np.arange(len(b_sorted)) - starts[b_sorted]
    total_tiles = int(tiles_per_win.sum())
    n = total_tiles * P
    # pad slots fetch the zeros row (row 0): repeated reads of one hot row
    # are DRAM row-buffer hits, cheaper than spreading pads randomly
    flat_idx = np.zeros((n_cores, n), dtype=np.int16)
    flat_dloc = np.full((n_cores, n), -1.0, dtype=np.float32)
    c = b_sorted // nw
    wloc = b_sorted % nw
    pos = win_tile_off[wloc] * P + rank
    flat_idx[c, pos] = order_vals_idx[mask][order]
    flat_dloc[c, pos] = order_vals_dloc[mask][order]
    idx16 = flat_idx.reshape(n_cores, n // 16, 16).transpose(0, 2, 1)
    idx16 = np.ascontiguousarray(np.tile(idx16, (1, 8, 1)))
    dl = np.ascontiguousarray(
        flat_dloc.reshape(n_cores, total_tiles, P).transpose(0, 2, 1)
    )
    return idx16, dl


def _bf16(a):
    import jax.numpy as jnp

    return np.asarray(jnp.asarray(np.asarray(a), dtype=jnp.bfloat16))


def kernel(x, edge_index, W):
    x = np.asarray(x, dtype=np.float32)
    edge_index = np.asarray(edge_index)
    W = np.asarray(W, dtype=np.float32)
    assert x.shape == (N_NODES, D) and W.shape == (D, D)

    src = edge_index[0].astype(np.int64)
    dst = edge_index[1].astype(np.int64)

    is_hi = src >= LO_CUT
    core = dst // DPC
    # per-core balanced window assignment
    lo_per_dst = np.bincount(dst[~is_hi], minlength=N_NODES)
    hi_per_dst = np.bincount(dst[is_hi], minlength=N_NODES)

    th = TH_DEFAULT
    t_hi_w, ho_tiles = _hi_tiles(th)
    hi_caps = t_hi_w * P

    w_of_dst = np.empty(N_NODES, np.int64)
    dloc_of_dst = np.empty(N_NODES, np.int64)
    max_lo = 0.0
    hi_ok = True
    for c in range(N_CORES):
        sl = slice(c * DPC, (c + 1) * DPC)
        win, dloc, wlo, whi = _balance(lo_per_dst[sl], hi_per_dst[sl], hi_caps)
        w_of_dst[sl] = win
        dloc_of_dst[sl] = dloc
        max_lo = max(max_lo, wlo.max())
        if np.any(whi > hi_caps):
            hi_ok = False
    if max_lo > CAP_LO * P or not hi_ok:
        # fallback: uniform generous capacities
        th = (5, 5, 5, 5, 5, 5, 5)
        t_hi_w, ho_tiles = _hi_tiles(th)
        hi_caps = t_hi_w * P
        t_lo = CAP_LO
        for c in range(N_CORES):
            sl = slice(c * DPC, (c + 1) * DPC)
            win, dloc, wlo, whi = _balance(lo_per_dst[sl], hi_per_dst[sl], hi_caps)
            w_of_dst[sl] = win
            dloc_of_dst[sl] = dloc
            t_lo = max(t_lo, int(np.ceil(wlo.max() / P)))
            t_hi_w = np.maximum(t_hi_w, int(np.ceil(whi.max() / P)))
        th = tuple(int(t_hi_w[c * CG]) for c in range(NCH))
        t_hi_w, ho_tiles = _hi_tiles(th)
    else:
        t_lo = CAP_LO

    lo_tiles_w = np.full(NW, t_lo, np.int64)
    lo_off = np.concatenate([[0], np.cumsum(lo_tiles_w)[:-1]])

    w_all = w_of_dst[dst]
    dloc_all = dloc_of_dst[dst].astype(np.float32)
    bucket = core * NW + w_all

    idx_val_lo = (src + 1).astype(np.int16, casting="unsafe")
    idx_val_hi = (src - LO_CUT + 1).astype(np.int16, casting="unsafe")
    idx16_lo, dloc_lo = _grid(bucket, ~is_hi, idx_val_lo, dloc_all,
                              lo_tiles_w, lo_off, LO_ROWS - 1)
    idx16_hi, dloc_hi = _grid(bucket, is_hi, idx_val_hi, dloc_all,
                              t_hi_w, ho_tiles, HI_ROWS - 1)

    tbl_lo = np.zeros((LO_ROWS, D), np.float32)
    tbl_lo[1:] = x[:LO_CUT]
    tbl_hi = np.zeros((HI_ROWS, D), np.float32)
    tbl_hi[1:] = x[LO_CUT:]
    tmax = max(t_lo, max(th))
    iota = np.tile(np.arange(P, dtype=np.float32), (P, tmax))
    iota = np.ascontiguousarray(iota)

    nc = _build_nc(t_lo, th)

    tbl_lo16 = _bf16(tbl_lo)
    tbl_hi16 = _bf16(tbl_hi)
    w16 = _bf16(W)
    iota16 = _bf16(iota)
    in_maps = []
    for c in range(N_CORES):
        in_maps.append(
            {
                "tbl_lo": tbl_lo16,
                "tbl_hi": tbl_hi16,
                "idx_lo": idx16_lo[c],
                "idx_hi": idx16_hi[c],
                "dloc_lo": _bf16(dloc_lo[c]),
                "dloc_hi": _bf16(dloc_hi[c]),
                "wmat": w16,
                "iota": iota16,
                # only read by the parts="compute" ablation variant
                "dummy_a": _bf16(np.full((P, CG * t_lo, P), 0.5, np.float32)),
                "dummy_b": _bf16(np.full((P, CG * max(th), P), 0.5, np.float32)),
            }
        )

    from concourse.bass_utils import run_bass_kernel_spmd

    res = run_bass_kernel_spmd(nc, in_maps, core_ids=list(range(N_CORES)))
    # stashed so a test harness can re-run / re-time this invocation
    global _LAST_RUN, _LAST_CAPS
    _LAST_RUN = (nc, in_maps)
    _LAST_CAPS = (t_lo, th)
    outs = np.stack([res.results[c]["out"] for c in range(N_CORES)])  # [8, NW*P, D]

    # unpermute: dst n lives at core n//DPC, row w_of_dst[n]*P + dloc_of_dst[n]
    full = outs[np.arange(N_NODES) // DPC, w_of_dst * P + dloc_of_dst]
    return np.ascontiguousarray(full)


_LAST_RUN = None
_LAST_CAPS = None
